# revision 10
# baseline (speedup 1.0000x reference)
"""CombinedSurvLoss (NLL survival + pairwise ranking) on 8 TRN2 NeuronCores.

Math
----
reference = mean_j L_j + 0.1 * total / count, where

  L_j     = -(1-c_j) * ln(clip(s_prev_j) * clip(h_j)) - 0.85 * c_j * ln(clip(s_now_j))
  total   = sum_{i,j} [c_i=0][Y_j>Y_i] relu(r_j - r_i),  r = hazards.sum(axis=1)
  count   = sum_{i,j} [c_i=0][Y_j>Y_i]

The O(B^2) ranking term is decomposed through per-class weight vectors
(K=4 classes):  p^a_i = [Y_i=a][c_i=0] (a<3),  q^b_i = [Y_i=b] (b>0).  With
V^{(u)}_x = sum_i u_i [r_x > r_i] (6 weighted-rank vectors sharing one
comparison matrix), one shows

  total = sum_x r_x * ( sum_{a<Y_x} V^{(p^a)}_x
                        - [c_x=0] * sum_{b>Y_x} (Q_b - V^{(q^b)}_x) )
  count = sum_{a<b} P_a Q_b          (P_a = sum p^a, Q_b = sum q^b)

On device the comparison tile C[i, x] = [r_x > r_i] for each 128-row i-block
is produced as bf16 on one of DVE (is_gt, 4x mode), ACT (Sign from the rb
PSUM accumulator, with halved weights plus a correction K precomputed from
U), or Pool (is_gt), and immediately contracted on the TensorEngine in
x-major form: for each 128-column x-chunk, out[x, u] += C[:, xc]^T @ U[b]
([128,128] weights x [128,6] moving — 6-column matmuls are nearly free and
land V directly in x-partition layout, so no transposes are needed in the
tail).  Each core owns a 1024-column x-slice and loops over all 64 i-blocks,
so V is complete per-core without a collective; the final scalar partial is
AllReduce-summed on device.

Sharding: x-slice (batch dim) of hazards/S/Y/c per core for the per-row
work; the full hazards/Y/c (bf16, ~96KB) are also DMA'd to every core so
each core can form all 64 i-blocks of the pairwise matrix (the "all-gather
of risk/Y/c" from the sharding hint, done host-side as input replication).
"""

import numpy as np

import concourse.mybir as mybir
import concourse.tile as tile
from concourse import bacc
from concourse.bass_utils import run_bass_kernel_spmd

F32 = mybir.dt.float32
BF16 = mybir.dt.bfloat16
AF = mybir.ActivationFunctionType
ALU = mybir.AluOpType
AX = mybir.AxisListType

NCORES = 8
B, K = 8192, 4
SH = B // NCORES          # 1024 rows per core
NBLK = B // 128           # 64 i-blocks (all rows, every core)
OWN = SH // 128           # 8 x-chunks of 128 in the core's own slice
CW = SH                   # compare width: the core's 1024 own columns
ALPHA = 0.15
RANKING_WEIGHT = 0.1
EPS = 1e-7

# Compare-work schedule: per i-block, the [128, CW] bf16 compare tile is
# produced by DVE (tensor_scalar is_gt, ~327ns), ACT (Sign activation off the
# rb PSUM halves, ~1038ns) or Pool (is_gt, ~1517ns).  Block-index ranges:
# ACT [0, N_ACT) (contiguous so the Sign halved-weight correction K is a
# plain reduce over U[:, 0:N_ACT, :]), Pool next, DVE the rest.  Execution
# order interleaves the engines so the in-order PSUM accumulation chain on
# PE never waits long on a slow tile.
N_ACT = 13
N_POOL = 8
N_DVE = NBLK - N_ACT - N_POOL


def _interleave(quota):
    acc = dict.fromkeys(quota, 0.0)
    total = sum(quota.values())
    out = []
    for _ in range(total):
        for k in quota:
            acc[k] += quota[k] / total
        pick = max(acc, key=lambda k: acc[k])
        acc[pick] -= 1.0
        out.append(pick)
    return out


def _mk_schedule():
    nxt = {"A": 0, "P": N_ACT, "D": N_ACT + N_POOL}
    sched = []
    for eng in _interleave({"D": N_DVE, "A": N_ACT, "P": N_POOL}):
        sched.append((eng, nxt[eng]))
        nxt[eng] += 1
    return sched


SCHEDULE = _mk_schedule()

DO_COLLECTIVE = True


def _build_program():
    nc = bacc.Bacc(
        "TRN2",
        target_bir_lowering=False,
        debug=False,
        enable_asserts=False,
        num_devices=NCORES,
    )

    # bf16 inputs: Y/c are small ints (exact in bf16); hazards in bf16 only
    # feed the compare/rank path (full f32 rows arrive in hs_own for the NLL
    # and the final r-weighting).
    hz_full = nc.dram_tensor("hz_full", [B, K], BF16, kind="ExternalInput").ap()
    yc_full = nc.dram_tensor("yc_full", [2, B], BF16, kind="ExternalInput").ap()
    hzT_own = nc.dram_tensor("hzT_own", [K, SH], BF16, kind="ExternalInput").ap()
    hs_own = nc.dram_tensor("hs_own", [2, SH, K], F32, kind="ExternalInput").ap()
    yc_own = nc.dram_tensor("yc_own", [2, SH], BF16, kind="ExternalInput").ap()
    out = nc.dram_tensor("out", [1, 1], F32, kind="ExternalOutput").ap()

    with tile.TileContext(nc) as tc:
        with (
            tc.tile_pool(name="const", bufs=1) as constp,
            tc.tile_pool(name="sb", bufs=1) as sb,
            tc.tile_pool(name="cmp", bufs=10) as cmpp,
            tc.tile_pool(name="ps", bufs=1, space="PSUM") as ps,
            tc.tile_pool(name="pst", bufs=1, space="PSUM") as pst,
            tc.tile_pool(name="psrb", bufs=2, space="PSUM") as psrb,
        ):
            # ---------- input loads ----------
            # SP queue order = latency priority: hzT (heads the rb chain),
            # the two hz halves (head the r_all chain), then own-row data.
            hzT = sb.tile([4, SH], BF16)
            nc.sync.dma_start(hzT[:], hzT_own)
            # full hazards [p, blk, k], global row i = p*NBLK + blk, in two
            # blk-halves so r_all[:, 0:32] is ready early
            hzp = sb.tile([128, NBLK, K], BF16)
            hz_re = hz_full.rearrange("(p b) k -> p b k", p=128)
            nc.sync.dma_start(hzp[:, 0:32], hz_re[:, 0:32])
            nc.sync.dma_start(hzp[:, 32:64], hz_re[:, 32:64])
            # own rows (f32) + own Y/c
            hso = sb.tile([128, 2, OWN, K], F32)
            nc.sync.dma_start(hso[:], hs_own.rearrange("t (b p) k -> p t b k", p=128))
            yco = sb.tile([128, 2, OWN], BF16)
            nc.sync.dma_start(yco[:], yc_own.rearrange("t (b p) -> p t b", p=128))
            yoi, coi = yco[:, 0, :], yco[:, 1, :]
            # full Y / c via the ACT queue (frees SP; lands ~2.7us for U-build)
            yc = sb.tile([128, 2, NBLK], BF16)
            nc.scalar.dma_start(yc[:], yc_full.rearrange("t (p b) -> p t b", p=128))
            yi, ci = yc[:, 0, :], yc[:, 1, :]

            # ---------- constants ----------
            ones4 = constp.tile([4, 128], BF16)
            nc.vector.memset(ones4[:], 1.0)
            ones1 = constp.tile([1, 128], BF16)
            nc.vector.memset(ones1[:], 1.0)
            onescol = constp.tile([128, 1], BF16)
            nc.vector.memset(onescol[:], 1.0)
            onescol_f = constp.tile([128, 1], F32)
            nc.vector.memset(onescol_f[:], 1.0)

            # Force the natural_log activation table set (contains BOTH Sign
            # and Ln) before any Sign compare, so the kernel needs exactly one
            # ACT table load.
            actwarm = sb.tile([1, 8], F32)
            nc.scalar.activation(actwarm[:], ones1[:, 0:8], AF.Ln)

            # ---------- rb: r_x broadcast to all partitions (PE ones-matmul)
            # kept in PSUM f32 (read directly by ACT Sign / copied for DVE)
            ps_rb0 = psrb.tile([128, 512], F32, tag="rb0")
            ps_rb1 = psrb.tile([128, 512], F32, tag="rb1")
            nc.tensor.matmul(
                ps_rb0[:], lhsT=ones4[:], rhs=hzT[:, 0:512], start=True, stop=True
            )
            nc.tensor.matmul(
                ps_rb1[:], lhsT=ones4[:], rhs=hzT[:, 512:1024], start=True, stop=True
            )
            rb = sb.tile([128, CW], BF16)
            nc.vector.tensor_copy(rb[:, 0:512], ps_rb0[:])
            nc.vector.tensor_copy(rb[:, 512:1024], ps_rb1[:])

            # r_all[p, blk] = full risk (compare scalars), halves as DMAs land
            r_all = sb.tile([128, NBLK], F32)
            nc.vector.tensor_reduce(r_all[:, 0:32], hzp[:, 0:32], axis=AX.X, op=ALU.add)
            nc.vector.tensor_reduce(r_all[:, 32:64], hzp[:, 32:64], axis=AX.X, op=ALU.add)
            neg_r = sb.tile([128, NBLK], F32)
            nc.vector.tensor_scalar(neg_r[:], r_all[:], -1.0, None, op0=ALU.mult)

            # ---------- weight matrix U[p, blk, u] (bf16) ----------
            # u 0..2 = p^a = [Y=a][c=0] (a=0,1,2); u 3..5 = q^b = [Y=b] (b=1,2,3)
            cbar = sb.tile([128, NBLK], BF16)  # 1 - c
            nc.gpsimd.tensor_scalar(cbar[:], ci, -1.0, 1.0, op0=ALU.mult, op1=ALU.add)
            U = sb.tile([128, NBLK, 6], BF16)
            tmp_eq = sb.tile([128, NBLK], BF16)
            for a in range(3):
                nc.gpsimd.tensor_scalar(
                    tmp_eq[:], yi, float(a), None, op0=ALU.is_equal
                )
                nc.gpsimd.tensor_tensor(U[:, :, a], tmp_eq[:], cbar[:], op=ALU.mult)
            for b in range(1, 4):
                nc.gpsimd.tensor_scalar(
                    U[:, :, 2 + b], yi, float(b), None, op0=ALU.is_equal
                )
            # halved weights for the ACT Sign blocks: sum u*(s+1)/2 =
            # sum (u/2)*s + K_u with K_u added in the tail
            Uh = sb.tile([128, N_ACT, 6], BF16)
            nc.vector.tensor_scalar(Uh[:], U[:, 0:N_ACT, :], 0.5, None, op0=ALU.mult)

            # ---------- own-row prep + NLL (wide ops; overlaps the loop) ----
            yof = sb.tile([128, OWN], F32)
            nc.vector.tensor_copy(yof[:], yoi)
            cobar = sb.tile([128, OWN], F32)  # 1 - c_own
            nc.vector.tensor_scalar(cobar[:], coi, -1.0, 1.0, op0=ALU.mult, op1=ALU.add)
            ro = sb.tile([128, OWN], F32)
            nc.vector.tensor_reduce(ro[:], hso[:, 0], axis=AX.X, op=ALU.add)

            # y-comparison masks for the tail
            gm = []
            for a in range(3):
                g = sb.tile([128, OWN], F32, tag=f"gm{a}")
                nc.vector.tensor_scalar(g[:], yof[:], float(a), None, op0=ALU.is_gt)
                gm.append(g)
            lm = {}
            for b in range(1, 4):
                l = sb.tile([128, OWN], F32, tag=f"lm{b}")
                nc.vector.tensor_scalar(l[:], yof[:], float(b), None, op0=ALU.is_lt)
                lm[b] = l

            # clip all own h/S at EPS, then one wide Ln for both
            hsc = sb.tile([128, 2, OWN, K], F32)
            nc.gpsimd.tensor_scalar(hsc[:], hso[:], EPS, None, op0=ALU.max)
            lnhs = sb.tile([128, 2, OWN, K], F32)
            nc.scalar.activation(lnhs[:], hsc[:], AF.Ln)
            lnh, lns = lnhs[:, 0], lnhs[:, 1]
            # G[:, :, k] = ln h_k + ln s_{k-1}  (ln s_{-1} = ln 1 = 0)
            G = sb.tile([128, OWN, K], F32)
            nc.gpsimd.tensor_copy(G[:], lnh)
            nc.gpsimd.tensor_tensor(
                G[:, :, 1:4], G[:, :, 1:4], lns[:, :, 0:3], op=ALU.add
            )
            # onehot E[:, :, k] = [Y_own = k]
            E = sb.tile([128, OWN, K], F32)
            iota4 = constp.tile([128, K], F32)
            for k in range(K):
                nc.gpsimd.memset(iota4[:, k : k + 1], float(k))
            yof3 = yof[:].broadcast_to([128, OWN, K])
            iota3 = iota4[:].rearrange("p (o k) -> p o k", o=1).broadcast_to(
                [128, OWN, K]
            )
            nc.vector.tensor_tensor(E[:], yof3, iota3, op=ALU.is_equal)
            # U_nll = sum_k E*G ; C_nll = sum_k E*lns
            EG = sb.tile([128, OWN, K], F32)
            nc.gpsimd.tensor_tensor(EG[:], E[:], G[:], op=ALU.mult)
            Unll = sb.tile([128, OWN], F32)
            nc.vector.tensor_reduce(Unll[:], EG[:], axis=AX.X, op=ALU.add)
            nc.gpsimd.tensor_tensor(EG[:], E[:], lns, op=ALU.mult)
            Cnll = sb.tile([128, OWN], F32)
            nc.vector.tensor_reduce(Cnll[:], EG[:], axis=AX.X, op=ALU.add)
            # L = -(cbar*(U - 0.85C) + 0.85C); minus sign folded into grand
            Lt = sb.tile([128, OWN], F32)
            nc.vector.scalar_tensor_tensor(
                Lt[:], Cnll[:], -(1.0 - ALPHA), Unll[:], op0=ALU.mult, op1=ALU.add
            )
            nc.vector.tensor_tensor(Lt[:], Lt[:], cobar[:], op=ALU.mult)
            t3 = sb.tile([128, OWN], F32)
            nc.vector.tensor_scalar(
                t3[:], Cnll[:], (1.0 - ALPHA), None, op0=ALU.mult
            )
            nc.vector.tensor_tensor(Lt[:], Lt[:], t3[:], op=ALU.add)

            # ---------- global per-class sums P/Q and Sign correction K ----
            # column sums of U via ones-matmul -> [1, blk*6] rows, then
            # strided reduces over blk; broadcast back down via ones1-matmul
            ps_pq = pst.tile([1, NBLK * 6], F32, tag="pq")
            nc.tensor.matmul(
                ps_pq[:], lhsT=onescol[:], rhs=U[:].rearrange("p b u -> p (b u)"),
                start=True, stop=True,
            )
            pqk_row = sb.tile([1, 12], F32)
            nc.vector.tensor_reduce(
                pqk_row[:, 0:6],
                ps_pq[:].rearrange("p (b u) -> p u b", u=6),
                axis=AX.X, op=ALU.add,
            )
            nc.vector.tensor_reduce(
                pqk_row[:, 6:12],
                ps_pq[:].rearrange("p (b u) -> p u b", u=6)[:, :, 0:N_ACT],
                axis=AX.X, op=ALU.add,
            )
            ps_bc = pst.tile([128, 12], F32, tag="bc")
            ones1_f = constp.tile([1, 128], F32)
            nc.vector.memset(ones1_f[:], 1.0)
            nc.tensor.matmul(
                ps_bc[:], lhsT=ones1_f[:], rhs=pqk_row[:], start=True, stop=True
            )
            QBK = sb.tile([128, 12], F32)  # [:,0:6]=P/Q, [:,6:12]=2K
            nc.vector.tensor_copy(QBK[:], ps_bc[:])
            KQ = sb.tile([128, 6], F32)  # K_u = half the ACT-range sum
            nc.vector.tensor_scalar(KQ[:], QBK[:, 6:12], 0.5, None, op0=ALU.mult)
            QmK = sb.tile([128, 6], F32)  # Q_u - K_u
            nc.vector.tensor_tensor(QmK[:], QBK[:, 0:6], KQ[:], op=ALU.subtract)

            # count = sum_{a<b} P_a Q_b -> rscale = 0.1/count
            sfx = sb.tile([128, 3], F32)
            nc.gpsimd.tensor_copy(sfx[:, 2:3], QBK[:, 5:6])
            nc.gpsimd.tensor_tensor(sfx[:, 1:2], QBK[:, 4:5], QBK[:, 5:6], op=ALU.add)
            nc.gpsimd.tensor_tensor(sfx[:, 0:1], QBK[:, 3:4], sfx[:, 1:2], op=ALU.add)
            cnt = sb.tile([128, 3], F32)
            nc.gpsimd.tensor_tensor(cnt[:], QBK[:, 0:3], sfx[:], op=ALU.mult)
            cnt1 = sb.tile([128, 1], F32)
            nc.vector.tensor_reduce(cnt1[:], cnt[:], axis=AX.X, op=ALU.add)
            rscale = sb.tile([128, 1], F32)
            nc.vector.reciprocal(rscale[:], cnt1[:])
            nc.vector.tensor_scalar(
                rscale[:], rscale[:], RANKING_WEIGHT, None, op0=ALU.mult
            )

            # ---------- main O(B^2) loop ----------
            # psV[x, xb*6+u] accumulates V over all i-blocks, x-major
            psV = ps.tile([128, OWN, 6], F32)
            last = len(SCHEDULE) - 1
            for it, (eng, b) in enumerate(SCHEDULE):
                C = cmpp.tile([128, CW], BF16, tag="C")
                if eng == "D":
                    nc.vector.tensor_scalar(
                        C[:], rb[:], r_all[:, b : b + 1], None, op0=ALU.is_gt
                    )
                elif eng == "A":
                    # reads the same bf16 rb as DVE/Pool so every pair's two
                    # compare directions use one consistent x-side rounding
                    nc.scalar.activation(
                        C[:], rb[:], AF.Sign, bias=neg_r[:, b : b + 1]
                    )
                else:
                    nc.gpsimd.tensor_scalar(
                        C[:], rb[:], r_all[:, b : b + 1], None, op0=ALU.is_gt
                    )
                rhs = Uh[:, b, :] if b < N_ACT else U[:, b, :]
                for xb in range(OWN):
                    # one accumulation group for the whole psV zero region:
                    # start zeroes the full 2KB region once, stop closes it
                    nc.tensor.matmul(
                        psV[:, xb, :],
                        lhsT=C[:, xb * 128 : (xb + 1) * 128],
                        rhs=rhs,
                        start=(it == 0 and xb == 0),
                        stop=(it == last and xb == OWN - 1),
                    )

            # ---------- tail: T1/T2 and the final scalar ----------
            Vt = sb.tile([128, OWN, 6], F32)
            nc.vector.tensor_copy(Vt[:], psV[:])

            # T1 = sum_{a<y} (V_a + K_a); T2' = sum_{b>y} (V_b - (Q_b - K_b))
            t1p = []
            for a in range(3):
                t = sb.tile([128, OWN], F32, tag=f"t1p{a}")
                nc.vector.scalar_tensor_tensor(
                    t[:], Vt[:, :, a], KQ[:, a : a + 1], gm[a][:],
                    op0=ALU.add, op1=ALU.mult,
                )
                t1p.append(t)
            t2p = []
            for b in range(1, 4):
                t = sb.tile([128, OWN], F32, tag=f"t2p{b}")
                nc.vector.scalar_tensor_tensor(
                    t[:], Vt[:, :, 2 + b], QmK[:, 2 + b : 3 + b], lm[b][:],
                    op0=ALU.subtract, op1=ALU.mult,
                )
                t2p.append(t)
            T1 = sb.tile([128, OWN], F32)
            T2 = sb.tile([128, OWN], F32)
            nc.vector.tensor_tensor(T1[:], t1p[0][:], t1p[1][:], op=ALU.add)
            nc.vector.tensor_tensor(T1[:], T1[:], t1p[2][:], op=ALU.add)
            nc.gpsimd.tensor_tensor(T2[:], t2p[0][:], t2p[1][:], op=ALU.add)
            nc.gpsimd.tensor_tensor(T2[:], T2[:], t2p[2][:], op=ALU.add)

            # contrib = r * (T1 + cbar * T2')   (T2' = -T2_true)
            contrib = sb.tile([128, OWN], F32)
            nc.vector.tensor_tensor(contrib[:], cobar[:], T2[:], op=ALU.mult)
            nc.vector.tensor_tensor(contrib[:], T1[:], contrib[:], op=ALU.add)
            nc.vector.tensor_tensor(contrib[:], contrib[:], ro[:], op=ALU.mult)

            # grand = -L/B + contrib * (0.1/count); reduce to a single scalar
            grand = sb.tile([128, OWN], F32)
            nc.vector.tensor_scalar(
                contrib[:], contrib[:], rscale[:, 0:1], None, op0=ALU.mult
            )
            red = sb.tile([128, 1], F32)
            nc.vector.scalar_tensor_tensor(
                grand[:], Lt[:], -1.0 / B, contrib[:],
                op0=ALU.mult, op1=ALU.add, accum_out=red[:],
            )
            ps_fin = pst.tile([1, 1], F32, tag="fin")
            nc.tensor.matmul(
                ps_fin[:], lhsT=red[:], rhs=onescol_f[:], start=True, stop=True
            )
            partial = sb.tile([1, 1], F32)
            nc.vector.tensor_copy(partial[:], ps_fin[:])

            # ---------- global sum ----------
            if DO_COLLECTIVE:
                with tc.tile_pool(name="dram", bufs=1, space="DRAM") as dramp:
                    cc_in = dramp.tile([1, 1], F32)
                    cc_out = dramp.tile([1, 1], F32)
                    nc.sync.dma_start(cc_in[:], partial[:])
                    nc.gpsimd.collective_compute(
                        "AllReduce",
                        ALU.add,
                        replica_groups=[list(range(NCORES))],
                        ins=[cc_in.opt()],
                        outs=[cc_out.opt()],
                    )
                    nc.sync.dma_start(out[:], cc_out[:])
            else:
                nc.sync.dma_start(out[:], partial[:])

    nc.compile()
    return nc


_PROGRAM = None


def _get_program():
    global _PROGRAM
    if _PROGRAM is None:
        _PROGRAM = _build_program()
    return _PROGRAM


def kernel(hazards, S, Y, c):
    hazards = np.ascontiguousarray(np.asarray(hazards, dtype=np.float32))
    S = np.ascontiguousarray(np.asarray(S, dtype=np.float32))
    import ml_dtypes

    bf16 = ml_dtypes.bfloat16
    hz_bf = hazards.astype(bf16)
    Yf = np.asarray(Y).astype(np.float32)
    cf = np.asarray(c).astype(np.float32)
    yc_full = np.ascontiguousarray(np.stack([Yf, cf]).astype(bf16))

    nc = _get_program()
    in_maps = []
    for m in range(NCORES):
        sl = slice(m * SH, (m + 1) * SH)
        in_maps.append(
            {
                "hz_full": hz_bf,
                "yc_full": yc_full,
                "hzT_own": np.ascontiguousarray(hz_bf[sl].T),
                "hs_own": np.ascontiguousarray(np.stack([hazards[sl], S[sl]])),
                "yc_own": np.ascontiguousarray(yc_full[:, sl]),
            }
        )
    res = run_bass_kernel_spmd(nc, in_maps, core_ids=list(range(NCORES)))
    if DO_COLLECTIVE:
        val = res.results[0]["out"][0, 0]
    else:
        val = np.float32(sum(r["out"][0, 0] for r in res.results))
    return np.asarray(val, dtype=np.float32).reshape(())
